# revision 8
# baseline (speedup 1.0000x reference)
"""Trainium2 Bass kernel for nn_AttentionGuidedGate (moe_routing).

Structure (8 NeuronCores, SPMD, no collectives):
  - Attention probe is HEAD-sharded: core c computes heads {2c, 2c+1} over all
    2048 tokens, producing a partial importance vector [2048] (summed on host).
  - Router MLP is TOKEN-sharded: core c computes tokens [256c, 256c+256).
  - Grouped top-1 routing == global argmax (the group holding the global max
    always survives top-2 group selection), so the router reduces to
    argmax(logits) and weight = 1/sum_j exp(l_j - l_max).

Host-side prep (part of input sharding):
  - x_projected / x are shipped pre-transposed (d-major) so projections need no
    on-device transpose.
  - attn_norm_w is folded into wq/wk (rmsnorm weight commutes into the matmul).
  - q_norm_w*k_norm_w/sqrt(128) is a per-head-dim scale folded into q on device.
  - bq = bk = 0 (asserted), which makes the x-rmsnorm scale cancel inside the
    per-head q/k rmsnorms (up to a ~1e-6 eps perturbation), so the x-norm is
    skipped entirely.

Probe dataflow per core (all tokens T=2048, heads m in {0,1} local):
  qT[d,t] = (wq' chunks).T @ xpT    (f32r matmuls, d on partitions)
  per-head ssq over d via ones-matmul -> rsq (per-token, as [128, 16] columns
  via a dram bounce + PE transpose) and rsk (per-token row, broadcast to
  [128, T] via a K=1 matmul)
  scores psum S[qt, kt] = qT(scaled by sqk).T @ kT(scaled by rsk)
  E = exp(S * rsq[qt])  (ACT, per-partition scale; no max subtraction, scores
  are O(1)); accum_out gives the softmax denominator D[qt]
  importance[kt] += sum_qt E[qt,kt]/D[qt] via matmul with lhsT = 1/D [128,1]
"""

import numpy as np

import concourse.bass as bass
import concourse.tile as tile
import concourse.mybir as mybir
from concourse import bacc
from concourse.bass import ts
from concourse.masks import make_identity

F32 = mybir.dt.float32
F32R = mybir.dt.float32r
BF16 = mybir.dt.bfloat16
U32 = mybir.dt.uint32
AF = mybir.ActivationFunctionType
MUL = None  # set lazily from AluOpType

NCORES = 8
T, AH, DIM, HID, E = 2048, 1024 * 2, 1024, 1024, 8
NH, D = 16, 128
HPC = NH // NCORES            # heads per core = 2
TLOC = T // NCORES            # router tokens per core = 256
NKC = AH // 128               # probe contraction chunks = 16
NQB = T // 128                # query blocks = 16
EPS = 1e-6
RECIP_HEADS = 1.0 / NH


def build_nc():
    from concourse.alu_op_type import AluOpType

    nc = bacc.Bacc("TRN2", target_bir_lowering=False, debug=False,
                   num_devices=NCORES)

    xpt = nc.declare_dram_parameter("xpt", [AH, T], F32R, isOutput=False)
    wq = nc.declare_dram_parameter("wq", [AH, HPC * D], F32R, isOutput=False)
    wk = nc.declare_dram_parameter("wk", [AH, HPC * D], F32R, isOutput=False)
    sqk = nc.declare_dram_parameter("sqk", [D], F32, isOutput=False)
    xt = nc.declare_dram_parameter("xt", [DIM, TLOC], F32, isOutput=False)
    w1 = nc.declare_dram_parameter("w1", [DIM, HID], F32, isOutput=False)
    b1 = nc.declare_dram_parameter("b1", [HID], F32, isOutput=False)
    w2 = nc.declare_dram_parameter("w2", [HID, HID], F32, isOutput=False)
    b2 = nc.declare_dram_parameter("b2", [HID], F32, isOutput=False)
    w3 = nc.declare_dram_parameter("w3", [HID, E], F32, isOutput=False)
    b3 = nc.declare_dram_parameter("b3", [E], F32, isOutput=False)
    impo = nc.declare_dram_parameter("imp", [T], F32, isOutput=True)
    rwo = nc.declare_dram_parameter("rw", [TLOC], F32, isOutput=True)
    rio = nc.declare_dram_parameter("ridx", [TLOC], U32, isOutput=True)

    ssq_q_d = nc.dram_tensor("ssq_q_d", [HPC, T], F32)

    with tile.TileContext(nc) as tc:
        with (tc.tile_pool(name="sbw", bufs=1) as sbw,
              tc.tile_pool(name="sbr", bufs=1) as sbr,
              tc.tile_pool(name="sbqk", bufs=1) as sbqk,
              tc.tile_pool(name="sbx", bufs=2) as sbx,
              tc.tile_pool(name="sbs", bufs=2) as sbs,
              tc.tile_pool(name="sbsq", bufs=16) as sbsq):

            # =================== router (token shard) ===================
            xt_s = sbr.tile([128, DIM // 128, TLOC], F32, tag="xt")
            nc.sync.dma_start(xt_s[:], xt[:, :].rearrange("(k p) n -> p k n", p=128))
            b1_s = sbr.tile([128, HID // 128], F32, tag="b1")
            nc.sync.dma_start(b1_s[:], b1[:].rearrange("(m p) -> p m", p=128))
            b2_s = sbr.tile([128, HID // 128], F32, tag="b2")
            nc.sync.dma_start(b2_s[:], b2[:].rearrange("(m p) -> p m", p=128))
            w3_s = sbr.tile([128, HID // 128, E], F32, tag="w3")
            nc.sync.dma_start(w3_s[:], w3[:, :].rearrange("(k p) e -> p k e", p=128))
            b3_s = sbr.tile([1, E], F32, tag="b3")
            nc.sync.dma_start(b3_s[:], b3[:])
            ones_col = sbr.tile([1, 128], F32, tag="onesc")
            nc.vector.memset(ones_col[:], 1.0)

            w1_s = sbw.tile([128, DIM // 128, HID], F32, tag="w12")
            nc.sync.dma_start(w1_s[:], w1[:, :].rearrange("(k p) m -> p k m", p=128))

            h1_s = sbr.tile([128, HID // 128, TLOC], F32, tag="h1")
            h2_s = sbr.tile([128, HID // 128, TLOC], F32, tag="h2")

            with tc.tile_pool(name="psr", bufs=2, space="PSUM") as psr:
                for m in range(HID // 128):
                    hp = psr.tile([128, TLOC], F32, tag="hp")
                    for k in range(DIM // 128):
                        nc.tensor.matmul(hp[:], w1_s[:, k, ts(m, 128)],
                                         xt_s[:, k, :],
                                         start=(k == 0), stop=(k == DIM // 128 - 1))
                    nc.scalar.activation(h1_s[:, m, :], hp[:], AF.Gelu,
                                         bias=b1_s[:, m:m + 1], scale=1.0)

                w2_s = sbw.tile([128, HID // 128, HID], F32, tag="w12")
                nc.sync.dma_start(w2_s[:],
                                  w2[:, :].rearrange("(k p) m -> p k m", p=128))
                for m in range(HID // 128):
                    hp = psr.tile([128, TLOC], F32, tag="hp")
                    for k in range(HID // 128):
                        nc.tensor.matmul(hp[:], w2_s[:, k, ts(m, 128)],
                                         h1_s[:, k, :],
                                         start=(k == 0), stop=(k == HID // 128 - 1))
                    nc.scalar.activation(h2_s[:, m, :], hp[:], AF.Gelu,
                                         bias=b2_s[:, m:m + 1], scale=1.0)

                for tb in range(TLOC // 128):
                    lp = psr.tile([128, E], F32, tag="lp")
                    for k in range(HID // 128):
                        nc.tensor.matmul(lp[:], h2_s[:, k, ts(tb, 128)],
                                         w3_s[:, k, :],
                                         start=(k == 0), stop=False)
                    nc.tensor.matmul(lp[:], ones_col[:], b3_s[:],
                                     start=False, stop=True)
                    lg_s = sbs.tile([128, E], F32, tag="lg")
                    nc.vector.tensor_copy(lg_s[:], lp[:])
                    e_s = sbs.tile([128, E], F32, tag="es")
                    esum = sbs.tile([128, 1], F32, tag="esum")
                    nc.scalar.activation(e_s[:], lp[:], AF.Exp, accum_out=esum[:])
                    mx8 = sbs.tile([128, 8], F32, tag="mx8")
                    nc.vector.max(mx8[:], lg_s[:])
                    midx = sbs.tile([128, 8], U32, tag="midx")
                    nc.vector.max_index(midx[:], mx8[:], lg_s[:])
                    emax = sbs.tile([128, 1], F32, tag="emax")
                    nc.scalar.activation(emax[:], mx8[:, 0:1], AF.Exp)
                    rec = sbs.tile([128, 1], F32, tag="rec")
                    nc.vector.reciprocal(rec[:], esum[:])
                    wout = sbs.tile([128, 1], F32, tag="wout")
                    nc.vector.tensor_mul(wout[:], emax[:], rec[:])
                    nc.sync.dma_start(rwo[ts(tb, 128)], wout[:])
                    nc.sync.dma_start(rio[ts(tb, 128)], midx[:, 0:1])

            # =================== probe projections (head shard) ===================
            wq_s = sbqk.tile([128, NKC, HPC * D], F32R, tag="wqk", bufs=2)
            nc.sync.dma_start(wq_s[:], wq[:, :].rearrange("(k p) m -> p k m", p=128))
            wk_s = sbqk.tile([128, NKC, HPC * D], F32R, tag="wqk", bufs=2)
            nc.sync.dma_start(wk_s[:], wk[:, :].rearrange("(k p) m -> p k m", p=128))
            sqk_s = sbqk.tile([128, 1], F32, tag="sqk")
            nc.sync.dma_start(sqk_s[:], sqk[:])

            qs = sbqk.tile([128, HPC, T], F32R, tag="qs")
            ks = sbqk.tile([128, HPC, T], F32R, tag="ks")

            sq_q = {}
            sq_k = {}
            with tc.tile_pool(name="psp", bufs=8, space="PSUM") as psp:
                for th in range(2):
                    prt = {}
                    for side in range(2):
                        for m in range(HPC):
                            for n in range(2):
                                prt[(side, m, n)] = psp.tile([128, 512], F32, tag="pr",
                                                             name=f"pr{th}_{side}{m}{n}")
                    for kc in range(NKC):
                        xc = sbx.tile([128, 1024], F32R, tag="xc")
                        nc.sync.dma_start(xc[:], xpt[ts(kc, 128), ts(th, 1024)])
                        for side, w_s in ((0, wq_s), (1, wk_s)):
                            for m in range(HPC):
                                for n in range(2):
                                    nc.tensor.matmul(
                                        prt[(side, m, n)][:],
                                        w_s[:, kc, ts(m, 128)],
                                        xc[:, ts(n, 512)],
                                        start=(kc == 0), stop=(kc == NKC - 1))
                    # drain + squares
                    for side in range(2):
                        dst = qs if side == 0 else ks
                        sqd = sq_q if side == 0 else sq_k
                        for m in range(HPC):
                            for n in range(2):
                                j = th * 2 + n
                                sl = dst[:, m, ts(j, 512)]
                                nc.vector.tensor_copy(sl, prt[(side, m, n)][:])
                                sqt = sbsq.tile([128, 512], BF16, tag="sq", bufs=16,
                                                name=f"sq{side}_{m}_{j}")
                                nc.vector.tensor_tensor(sqt[:], sl, sl,
                                                        op=AluOpType.mult)
                                sqd[(m, j)] = sqt
                # fold per-d scale (q_norm_w*k_norm_w/sqrt(D)) into q, in place
                for m in range(HPC):
                    nc.vector.tensor_scalar_mul(qs[:, m, :], qs[:, m, :],
                                                sqk_s[:])

            # =================== per-head rms stats ===================
            rsqc = {}
            with (tc.tile_pool(name="pss", bufs=2, space="PSUM") as pss,
                  tc.tile_pool(name="pst", bufs=2, space="PSUM") as pst):
                ones_f32 = sbqk.tile([128, 1], F32, tag="of32")
                nc.vector.memset(ones_f32[:], 1.0)
                ones_row_f32 = sbqk.tile([1, 128], F32, tag="orf32")
                nc.vector.memset(ones_row_f32[:], 1.0)
                ones_col_r = sbqk.tile([128, 1], BF16, tag="ocr")
                nc.vector.tensor_copy(ones_col_r[:], ones_f32[:])
                ones_row_r = sbqk.tile([1, 128], F32R, tag="orr")
                with nc.allow_low_precision(reason="exact 1.0 round to f32r"):
                    nc.vector.tensor_copy(ones_row_r[:], ones_row_f32[:])
                ident = sbqk.tile([128, 128], F32, tag="ident")
                make_identity(nc, ident[:])
                eps_col = sbqk.tile([128, 1], F32, tag="epsc")
                nc.vector.memset(eps_col[:], EPS)

                for m in range(HPC):
                    # q side: rsq as [128, NQB] columns (token-major partitions)
                    for j in range(4):
                        sp = pss.tile([1, 512], F32, tag="sp")
                        nc.tensor.matmul(sp[:], ones_col_r[:], sq_q[(m, j)][:],
                                         start=True, stop=True)
                        srow = sbs.tile([1, 512], F32, tag="srow",
                                        name=f"srow{m}_{j}")
                        nc.vector.tensor_copy(srow[:], sp[:])
                        nc.sync.dma_start(ssq_q_d[m, ts(j, 512)], srow[:])
                    q16 = sbs.tile([16, 128], F32, tag="q16")
                    nc.sync.dma_start(q16[:],
                                      ssq_q_d[m, :].rearrange("(b c) -> b c", c=128))
                    q16s = sbs.tile([16, 128], F32, tag="q16s")
                    nc.scalar.activation(q16s[:], q16[:], AF.Sqrt,
                                         bias=eps_col[:16, :], scale=1.0 / D)
                    tp = pst.tile([128, 16], F32, tag="tp")
                    nc.tensor.transpose(tp[:], q16s[:], ident[:16, :16])
                    rq = sbqk.tile([128, NQB], F32, tag=f"rsq{m}", name=f"rsq{m}")
                    nc.vector.reciprocal(rq[:], tp[:])
                    rsqc[m] = rq

                    # k side: rsk row -> broadcast -> scale ks in place
                    krow = sbs.tile([1, T], F32, tag="krow", bufs=1, name=f"krow{m}")
                    rkrow = sbs.tile([1, T], F32R, tag="rkrow", bufs=1, name=f"rkrow{m}")
                    for j in range(4):
                        sp = pss.tile([1, 512], F32, tag="sp")
                        nc.tensor.matmul(sp[:], ones_col_r[:], sq_k[(m, j)][:],
                                         start=True, stop=True)
                        nc.scalar.activation(krow[:, ts(j, 512)], sp[:], AF.Sqrt,
                                             bias=eps_col[:1, :], scale=1.0 / D)
                    with nc.allow_low_precision(reason="f32r rounding for matmul rhs"):
                        nc.vector.reciprocal(rkrow[:], krow[:])
                    rskb = sbs.tile([128, T], F32, tag="rskb", bufs=1, name=f"rskb{m}")
                    for j in range(4):
                        bp = pst.tile([128, 512], F32, tag="bp")
                        nc.tensor.matmul(bp[:], ones_row_r[:],
                                         rkrow[:, ts(j, 512)],
                                         start=True, stop=True)
                        nc.vector.tensor_copy(rskb[:, ts(j, 512)], bp[:])
                    nc.vector.tensor_tensor(ks[:, m, :], ks[:, m, :], rskb[:],
                                            op=AluOpType.mult)

            # =================== scores + softmax + importance ===================
            with (tc.tile_pool(name="psS", bufs=2, space="PSUM") as psS,
                  tc.tile_pool(name="psI", bufs=1, space="PSUM") as psI):
                impp = psI.tile([1, T], F32, tag="impp")
                iters = [(m, qb) for m in range(HPC) for qb in range(NQB)]
                pend = None   # (E tile, r tile) deferred for PE pipelining
                for it, (m, qb) in enumerate(iters):
                    Et = sbqk.tile([128, T], F32R, tag="wqk", bufs=2, name=f"E{it}")
                    dh = sbs.tile([128, 2], F32, tag="dh")
                    for kh in range(2):
                        Sp = psS.tile([128, 1024], F32, tag="S")
                        for n in range(2):
                            nc.tensor.matmul(
                                Sp[:, ts(n, 512)],
                                qs[:, m, ts(qb, 128)],
                                ks[:, m, bass.ds(kh * 1024 + n * 512, 512)],
                                start=True, stop=True)
                        nc.scalar.activation(Et[:, ts(kh, 1024)], Sp[:], AF.Exp,
                                             scale=rsqc[m][:, qb:qb + 1],
                                             accum_out=dh[:, kh:kh + 1])
                    Dt = sbs.tile([128, 1], F32, tag="D")
                    nc.vector.tensor_add(Dt[:], dh[:, 0:1], dh[:, 1:2])
                    rt = sbs.tile([128, 1], F32R, tag="r")
                    with nc.allow_low_precision(reason="f32r rounding for matmul lhsT"):
                        nc.vector.reciprocal(rt[:], Dt[:])
                    if pend is not None:
                        pE, pr, pit = pend
                        for n in range(4):
                            nc.tensor.matmul(impp[0:1, ts(n, 512)], pr[:],
                                             pE[:, ts(n, 512)],
                                             start=(pit == 0), stop=False)
                    pend = (Et, rt, it)
                pE, pr, pit = pend
                for n in range(4):
                    nc.tensor.matmul(impp[0:1, ts(n, 512)], pr[:],
                                     pE[:, ts(n, 512)],
                                     start=False, stop=True)

                imp_row = sbs.tile([1, T], F32, tag="improw", bufs=1)
                nc.vector.tensor_scalar_mul(imp_row[:], impp[0:1, :],
                                            RECIP_HEADS)
                nc.sync.dma_start(impo[:], imp_row[:])

    nc.compile()
    return nc


_NC_CACHE = None


def _get_nc():
    global _NC_CACHE
    if _NC_CACHE is None:
        _NC_CACHE = build_nc()
    return _NC_CACHE


def build_in_maps(inputs):
    f = lambda k: np.ascontiguousarray(np.asarray(inputs[k], dtype=np.float32))
    x = f("x")
    xp = f("x_projected")
    anw = f("attn_norm_w")
    wq_in = f("wq")
    wk_in = f("wk")
    bq = f("bq")
    bk = f("bk")
    qnw = f("q_norm_w")
    knw = f("k_norm_w")
    w1 = f("w1")
    b1 = f("b1")
    w2 = f("w2")
    b2 = f("b2")
    w3 = f("w3")
    b3 = f("b3")
    assert not np.any(bq) and not np.any(bk), \
        "kernel assumes zero q/k biases (x-rmsnorm cancellation)"

    wqf = anw[:, None] * wq_in
    wkf = anw[:, None] * wk_in
    sqk = (qnw * knw / np.sqrt(D)).astype(np.float32)
    xpt = np.ascontiguousarray(xp.T)
    xtf = np.ascontiguousarray(x.T)

    in_maps = []
    for c in range(NCORES):
        in_maps.append({
            "xpt": xpt,
            "wq": np.ascontiguousarray(wqf[:, c * HPC * D:(c + 1) * HPC * D]),
            "wk": np.ascontiguousarray(wkf[:, c * HPC * D:(c + 1) * HPC * D]),
            "sqk": sqk,
            "xt": np.ascontiguousarray(xtf[:, c * TLOC:(c + 1) * TLOC]),
            "w1": w1, "b1": b1, "w2": w2, "b2": b2, "w3": w3, "b3": b3,
        })
    return in_maps


def postprocess(results):
    weights = np.concatenate([results[c]["rw"] for c in range(NCORES)])
    weights = weights.astype(np.float32).reshape(T, 1)
    idx = np.concatenate([results[c]["ridx"] for c in range(NCORES)])
    idx = idx.astype(np.int32).reshape(T, 1)
    importance = np.zeros(T, dtype=np.float64)
    for c in range(NCORES):
        importance += results[c]["imp"].astype(np.float64)
    return weights, idx, importance.astype(np.float32)


def kernel(**inputs):
    from concourse.bass_utils import run_bass_kernel_spmd
    nc = _get_nc()
    in_maps = build_in_maps(inputs)
    res = run_bass_kernel_spmd(nc, in_maps, core_ids=list(range(NCORES)))
    return postprocess(res.results)


# AluOpType import at module scope for the builder
from concourse.alu_op_type import AluOpType  # noqa: E402


# revision 9
# speedup vs baseline: 1.2168x; 1.2168x over previous
"""Trainium2 Bass kernel for nn_AttentionGuidedGate (moe_routing).

Structure (8 NeuronCores, SPMD, no collectives):
  - Attention probe is HEAD-sharded: core c computes heads {2c, 2c+1} over all
    2048 tokens, producing a partial importance vector [2048] (summed on host).
  - Router MLP is TOKEN-sharded: core c computes tokens [256c, 256c+256).
  - Grouped top-1 routing == global argmax (the group holding the global max
    always survives top-2 group selection), so the router reduces to
    argmax(logits) and weight = 1/sum_j exp(l_j - l_max).

Host-side prep (part of input sharding):
  - x_projected / x are shipped pre-transposed (d-major) so projections need no
    on-device transpose.
  - attn_norm_w is folded into wq/wk (rmsnorm weight commutes into the matmul).
  - q_norm_w*k_norm_w/sqrt(128) is a per-head-dim scale folded into q on device.
  - bq = bk = 0 (asserted), which makes the x-rmsnorm scale cancel inside the
    per-head q/k rmsnorms (up to a ~1e-6 eps perturbation), so the x-norm is
    skipped entirely.

Probe dataflow per core (all tokens T=2048, heads m in {0,1} local):
  qT[d,t] = (wq' chunks).T @ xpT    (f32r matmuls, d on partitions)
  per-head ssq over d via ones-matmul -> rsq (per-token, as [128, 16] columns
  via a dram bounce + PE transpose) and rsk (per-token row, broadcast to
  [128, T] via a K=1 matmul)
  scores psum S[qt, kt] = qT(scaled by sqk).T @ kT(scaled by rsk)
  E = exp(S * rsq[qt])  (ACT, per-partition scale; no max subtraction, scores
  are O(1)); accum_out gives the softmax denominator D[qt]
  importance[kt] += sum_qt E[qt,kt]/D[qt] via matmul with lhsT = 1/D [128,1]
"""

import numpy as np

import concourse.bass as bass
import concourse.tile as tile
import concourse.mybir as mybir
from concourse import bacc
from concourse.bass import ts
from concourse.masks import make_identity

F32 = mybir.dt.float32
F32R = mybir.dt.float32r
BF16 = mybir.dt.bfloat16
U32 = mybir.dt.uint32
AF = mybir.ActivationFunctionType
MUL = None  # set lazily from AluOpType

NCORES = 8
T, AH, DIM, HID, E = 2048, 1024 * 2, 1024, 1024, 8
NH, D = 16, 128
HPC = NH // NCORES            # heads per core = 2
TLOC = T // NCORES            # router tokens per core = 256
NKC = AH // 128               # probe contraction chunks = 16
NQB = T // 128                # query blocks = 16
EPS = 1e-6
RECIP_HEADS = 1.0 / NH


def build_nc():
    from concourse.alu_op_type import AluOpType

    nc = bacc.Bacc("TRN2", target_bir_lowering=False, debug=False,
                   num_devices=NCORES)

    xpt = nc.declare_dram_parameter("xpt", [AH, T], F32R, isOutput=False)
    wq = nc.declare_dram_parameter("wq", [AH, HPC * D], F32R, isOutput=False)
    wk = nc.declare_dram_parameter("wk", [AH, HPC * D], F32R, isOutput=False)
    sqk = nc.declare_dram_parameter("sqk", [D], F32, isOutput=False)
    xt = nc.declare_dram_parameter("xt", [DIM, TLOC], F32, isOutput=False)
    w1 = nc.declare_dram_parameter("w1", [DIM, HID], F32, isOutput=False)
    b1 = nc.declare_dram_parameter("b1", [HID], F32, isOutput=False)
    w2 = nc.declare_dram_parameter("w2", [HID, HID], F32, isOutput=False)
    b2 = nc.declare_dram_parameter("b2", [HID], F32, isOutput=False)
    w3 = nc.declare_dram_parameter("w3", [HID, E], F32, isOutput=False)
    b3 = nc.declare_dram_parameter("b3", [E], F32, isOutput=False)
    impo = nc.declare_dram_parameter("imp", [T], F32, isOutput=True)
    rwo = nc.declare_dram_parameter("rw", [TLOC], F32, isOutput=True)
    rio = nc.declare_dram_parameter("ridx", [TLOC], U32, isOutput=True)

    ssq_q_d = nc.dram_tensor("ssq_q_d", [HPC, T], F32)

    with tile.TileContext(nc) as tc:
        with (tc.tile_pool(name="sbw", bufs=1) as sbw,
              tc.tile_pool(name="sbr", bufs=1) as sbr,
              tc.tile_pool(name="sbqk", bufs=1) as sbqk,
              tc.tile_pool(name="sbx", bufs=2) as sbx,
              tc.tile_pool(name="sbs", bufs=2) as sbs,
              tc.tile_pool(name="sbsq", bufs=16) as sbsq):

            # ---- shared setup DMAs (issued early; all overlap compute) ----
            xt_s = sbr.tile([128, DIM // 128, TLOC], F32, tag="xt")
            nc.sync.dma_start(xt_s[:], xt[:, :].rearrange("(k p) n -> p k n", p=128))
            b1_s = sbr.tile([128, HID // 128], F32, tag="b1")
            nc.sync.dma_start(b1_s[:], b1[:].rearrange("(m p) -> p m", p=128))
            b2_s = sbr.tile([128, HID // 128], F32, tag="b2")
            nc.sync.dma_start(b2_s[:], b2[:].rearrange("(m p) -> p m", p=128))
            w3_s = sbr.tile([128, HID // 128, E], F32, tag="w3")
            nc.sync.dma_start(w3_s[:], w3[:, :].rearrange("(k p) e -> p k e", p=128))
            b3_s = sbr.tile([1, E], F32, tag="b3")
            nc.sync.dma_start(b3_s[:], b3[:])
            ones_col = sbr.tile([1, 128], F32, tag="onesc")
            nc.vector.memset(ones_col[:], 1.0)

            wq_s = sbqk.tile([128, NKC, HPC * D], F32R, tag="wqk", bufs=2)
            nc.sync.dma_start(wq_s[:], wq[:, :].rearrange("(k p) m -> p k m", p=128))
            wk_s = sbqk.tile([128, NKC, HPC * D], F32R, tag="wqk", bufs=2)
            nc.sync.dma_start(wk_s[:], wk[:, :].rearrange("(k p) m -> p k m", p=128))
            sqk_s = sbqk.tile([128, 1], F32, tag="sqk")
            nc.sync.dma_start(sqk_s[:], sqk[:])

            w1_s = sbw.tile([128, DIM // 128, HID], F32, tag="w12")
            nc.sync.dma_start(w1_s[:], w1[:, :].rearrange("(k p) m -> p k m", p=128))

            ones_f32 = sbqk.tile([128, 1], F32, tag="of32")
            nc.vector.memset(ones_f32[:], 1.0)
            ones_row_f32 = sbqk.tile([1, 128], F32, tag="orf32")
            nc.vector.memset(ones_row_f32[:], 1.0)
            ones_col_r = sbqk.tile([128, 1], BF16, tag="ocr")
            nc.vector.tensor_copy(ones_col_r[:], ones_f32[:])
            ones_row_r = sbqk.tile([1, 128], F32R, tag="orr")
            with nc.allow_low_precision(reason="exact 1.0 round to f32r"):
                nc.vector.tensor_copy(ones_row_r[:], ones_row_f32[:])
            ident = sbqk.tile([128, 128], F32, tag="ident")
            make_identity(nc, ident[:])
            eps_col = sbqk.tile([128, 1], F32, tag="epsc")
            nc.vector.memset(eps_col[:], EPS)

            qs = sbqk.tile([128, HPC, T], F32R, tag="qs")
            ks = sbqk.tile([128, HPC, T], F32R, tag="ks")

            h1_s = sbr.tile([128, HID // 128, TLOC], F32, tag="h1")
            h2_s = sbr.tile([128, HID // 128, TLOC], F32, tag="h2")

            # =================== probe projections (head shard) ===================
            sq_q = {}
            sq_k = {}
            with tc.tile_pool(name="psp", bufs=8, space="PSUM") as psp:
                for th in range(2):
                    prt = {}
                    for side in range(2):
                        for m in range(HPC):
                            for n in range(2):
                                prt[(side, m, n)] = psp.tile([128, 512], F32, tag="pr",
                                                             name=f"pr{th}_{side}{m}{n}")
                    for kc in range(NKC):
                        xc = sbx.tile([128, 1024], F32R, tag="xc", bufs=3,
                                      name=f"xc{th}_{kc}")
                        nc.sync.dma_start(xc[:], xpt[ts(kc, 128), ts(th, 1024)])
                        for side, w_s in ((0, wq_s), (1, wk_s)):
                            for m in range(HPC):
                                for n in range(2):
                                    nc.tensor.matmul(
                                        prt[(side, m, n)][:],
                                        w_s[:, kc, ts(m, 128)],
                                        xc[:, ts(n, 512)],
                                        start=(kc == 0), stop=(kc == NKC - 1))
                    # drain + squares
                    for side in range(2):
                        dst = qs if side == 0 else ks
                        sqd = sq_q if side == 0 else sq_k
                        for m in range(HPC):
                            for n in range(2):
                                j = th * 2 + n
                                sl = dst[:, m, ts(j, 512)]
                                nc.vector.tensor_copy(sl, prt[(side, m, n)][:])
                                sqt = sbsq.tile([128, 512], BF16, tag="sq", bufs=16,
                                                name=f"sq{side}_{m}_{j}")
                                nc.vector.tensor_tensor(sqt[:], sl, sl,
                                                        op=AluOpType.mult)
                                sqd[(m, j)] = sqt
                # fold per-d scale (q_norm_w*k_norm_w/sqrt(D)) into q, in place
                for m in range(HPC):
                    nc.vector.tensor_scalar_mul(qs[:, m, :], qs[:, m, :],
                                                sqk_s[:])

            # ============ rms stats (PE-light) interleaved with router ============
            rsqc = {}
            with (tc.tile_pool(name="psr", bufs=2, space="PSUM") as psr,
                  tc.tile_pool(name="pss", bufs=2, space="PSUM") as pss,
                  tc.tile_pool(name="pst", bufs=2, space="PSUM") as pst):
                # -- all ssq ones-matmuls first (quick PE work) --
                krows = {}
                for m in range(HPC):
                    for j in range(4):
                        sp = pss.tile([1, 512], F32, tag="sp",
                                      name=f"spq{m}_{j}")
                        nc.tensor.matmul(sp[:], ones_col_r[:], sq_q[(m, j)][:],
                                         start=True, stop=True)
                        srow = sbs.tile([1, 512], F32, tag="srow",
                                        name=f"srow{m}_{j}")
                        nc.vector.tensor_copy(srow[:], sp[:])
                        nc.sync.dma_start(ssq_q_d[m, ts(j, 512)], srow[:])
                for m in range(HPC):
                    krow = sbs.tile([1, T], F32, tag="krow", bufs=1,
                                    name=f"krow{m}")
                    rkrow = sbs.tile([1, T], F32R, tag="rkrow", bufs=1,
                                     name=f"rkrow{m}")
                    krows[m] = (krow, rkrow)
                    for j in range(4):
                        sp = pss.tile([1, 512], F32, tag="sp",
                                      name=f"spk{m}_{j}")
                        nc.tensor.matmul(sp[:], ones_col_r[:], sq_k[(m, j)][:],
                                         start=True, stop=True)
                        nc.scalar.activation(krow[:, ts(j, 512)], sp[:], AF.Sqrt,
                                             bias=eps_col[:1, :], scale=1.0 / D)
                        with nc.allow_low_precision(reason="f32r for matmul rhs"):
                            nc.vector.reciprocal(rkrow[:, ts(j, 512)],
                                                 krow[:, ts(j, 512)])

                # -- router layer 1 (fills PE while stats chains run) --
                for m in range(HID // 128):
                    hp = psr.tile([128, TLOC], F32, tag="hp", name=f"h1p{m}")
                    for k in range(DIM // 128):
                        nc.tensor.matmul(hp[:], w1_s[:, k, ts(m, 128)],
                                         xt_s[:, k, :],
                                         start=(k == 0), stop=(k == DIM // 128 - 1))
                    nc.scalar.activation(h1_s[:, m, :], hp[:], AF.Gelu,
                                         bias=b1_s[:, m:m + 1], scale=1.0)
                w2_s = sbw.tile([128, HID // 128, HID], F32, tag="w12")
                nc.sync.dma_start(w2_s[:],
                                  w2[:, :].rearrange("(k p) m -> p k m", p=128))

                # -- stats tail: transposes + broadcasts (PE-tiny) --
                for m in range(HPC):
                    q16 = sbs.tile([16, 128], F32, tag="q16", name=f"q16_{m}")
                    nc.sync.dma_start(q16[:],
                                      ssq_q_d[m, :].rearrange("(b c) -> b c", c=128))
                    q16s = sbs.tile([16, 128], F32, tag="q16s", name=f"q16s{m}")
                    nc.scalar.activation(q16s[:], q16[:], AF.Sqrt,
                                         bias=eps_col[:16, :], scale=1.0 / D)
                    tp = pst.tile([128, 16], F32, tag="tp", name=f"tp{m}")
                    nc.tensor.transpose(tp[:], q16s[:], ident[:16, :16])
                    rq = sbqk.tile([128, NQB], F32, tag=f"rsq{m}", name=f"rsq{m}")
                    nc.vector.reciprocal(rq[:], tp[:])
                    rsqc[m] = rq

                    rkrow = krows[m][1]
                    rskb = sbs.tile([128, T], F32, tag="rskb", bufs=1,
                                    name=f"rskb{m}")
                    for j in range(4):
                        bp = pst.tile([128, 512], F32, tag="bp", name=f"bp{m}_{j}")
                        nc.tensor.matmul(bp[:], ones_row_r[:],
                                         rkrow[:, ts(j, 512)],
                                         start=True, stop=True)
                        nc.vector.tensor_copy(rskb[:, ts(j, 512)], bp[:])
                    nc.vector.tensor_tensor(ks[:, m, :], ks[:, m, :], rskb[:],
                                            op=AluOpType.mult)

                # -- router layer 2 + logits --
                for m in range(HID // 128):
                    hp = psr.tile([128, TLOC], F32, tag="hp", name=f"h2p{m}")
                    for k in range(HID // 128):
                        nc.tensor.matmul(hp[:], w2_s[:, k, ts(m, 128)],
                                         h1_s[:, k, :],
                                         start=(k == 0), stop=(k == HID // 128 - 1))
                    nc.scalar.activation(h2_s[:, m, :], hp[:], AF.Gelu,
                                         bias=b2_s[:, m:m + 1], scale=1.0)

                for tb in range(TLOC // 128):
                    lp = psr.tile([128, E], F32, tag="hp", name=f"lp{tb}")
                    for k in range(HID // 128):
                        nc.tensor.matmul(lp[:, 0:E], h2_s[:, k, ts(tb, 128)],
                                         w3_s[:, k, :],
                                         start=(k == 0), stop=False)
                    nc.tensor.matmul(lp[:, 0:E], ones_col[:], b3_s[:],
                                     start=False, stop=True)
                    lg_s = sbs.tile([128, E], F32, tag="lg", name=f"lg{tb}")
                    nc.vector.tensor_copy(lg_s[:], lp[:, 0:E])
                    e_s = sbs.tile([128, E], F32, tag="es", name=f"es{tb}")
                    esum = sbs.tile([128, 1], F32, tag="esum", name=f"esum{tb}")
                    nc.scalar.activation(e_s[:], lp[:, 0:E], AF.Exp,
                                         accum_out=esum[:])
                    mx8 = sbs.tile([128, 8], F32, tag="mx8", name=f"mx8_{tb}")
                    nc.vector.max(mx8[:], lg_s[:])
                    midx = sbs.tile([128, 8], U32, tag="midx", name=f"midx{tb}")
                    nc.vector.max_index(midx[:], mx8[:], lg_s[:])
                    emax = sbs.tile([128, 1], F32, tag="emax", name=f"emax{tb}")
                    nc.scalar.activation(emax[:], mx8[:, 0:1], AF.Exp)
                    rec = sbs.tile([128, 1], F32, tag="rec", name=f"rec{tb}")
                    nc.vector.reciprocal(rec[:], esum[:])
                    wout = sbs.tile([128, 1], F32, tag="wout", name=f"wout{tb}")
                    nc.vector.tensor_mul(wout[:], emax[:], rec[:])
                    nc.sync.dma_start(rwo[ts(tb, 128)], wout[:])
                    nc.sync.dma_start(rio[ts(tb, 128)], midx[:, 0:1])

            # =================== scores + softmax + importance ===================
            with (tc.tile_pool(name="psS", bufs=2, space="PSUM") as psS,
                  tc.tile_pool(name="psI", bufs=1, space="PSUM") as psI):
                impp = psI.tile([1, T], F32, tag="impp")
                iters = [(m, qb) for m in range(HPC) for qb in range(NQB)]
                pend = None   # (E tile, r tile) deferred for PE pipelining
                for it, (m, qb) in enumerate(iters):
                    Et = sbqk.tile([128, T], F32R, tag="wqk", bufs=2, name=f"E{it}")
                    dh = sbs.tile([128, 2], F32, tag="dh", name=f"dh{it}")
                    for kh in range(2):
                        Sp = psS.tile([128, 1024], F32, tag="S", name=f"S{it}_{kh}")
                        for n in range(2):
                            nc.tensor.matmul(
                                Sp[:, ts(n, 512)],
                                qs[:, m, ts(qb, 128)],
                                ks[:, m, bass.ds(kh * 1024 + n * 512, 512)],
                                start=True, stop=True)
                        nc.scalar.activation(Et[:, ts(kh, 1024)], Sp[:], AF.Exp,
                                             scale=rsqc[m][:, qb:qb + 1],
                                             accum_out=dh[:, kh:kh + 1])
                    Dt = sbs.tile([128, 1], F32, tag="D", name=f"D{it}")
                    nc.vector.tensor_add(Dt[:], dh[:, 0:1], dh[:, 1:2])
                    rt = sbs.tile([128, 1], F32R, tag="r", name=f"r{it}")
                    with nc.allow_low_precision(reason="f32r rounding for matmul lhsT"):
                        nc.vector.reciprocal(rt[:], Dt[:])
                    if pend is not None:
                        pE, pr, pit = pend
                        for n in range(4):
                            nc.tensor.matmul(impp[0:1, ts(n, 512)], pr[:],
                                             pE[:, ts(n, 512)],
                                             start=(pit == 0), stop=False)
                    pend = (Et, rt, it)
                pE, pr, pit = pend
                for n in range(4):
                    nc.tensor.matmul(impp[0:1, ts(n, 512)], pr[:],
                                     pE[:, ts(n, 512)],
                                     start=False, stop=True)

                imp_row = sbs.tile([1, T], F32, tag="improw", bufs=1)
                nc.vector.tensor_scalar_mul(imp_row[:], impp[0:1, :],
                                            RECIP_HEADS)
                nc.sync.dma_start(impo[:], imp_row[:])

    nc.compile()
    return nc


_NC_CACHE = None


def _get_nc():
    global _NC_CACHE
    if _NC_CACHE is None:
        _NC_CACHE = build_nc()
    return _NC_CACHE


def build_in_maps(inputs):
    f = lambda k: np.ascontiguousarray(np.asarray(inputs[k], dtype=np.float32))
    x = f("x")
    xp = f("x_projected")
    anw = f("attn_norm_w")
    wq_in = f("wq")
    wk_in = f("wk")
    bq = f("bq")
    bk = f("bk")
    qnw = f("q_norm_w")
    knw = f("k_norm_w")
    w1 = f("w1")
    b1 = f("b1")
    w2 = f("w2")
    b2 = f("b2")
    w3 = f("w3")
    b3 = f("b3")
    assert not np.any(bq) and not np.any(bk), \
        "kernel assumes zero q/k biases (x-rmsnorm cancellation)"

    wqf = anw[:, None] * wq_in
    wkf = anw[:, None] * wk_in
    sqk = (qnw * knw / np.sqrt(D)).astype(np.float32)
    xpt = np.ascontiguousarray(xp.T)
    xtf = np.ascontiguousarray(x.T)

    in_maps = []
    for c in range(NCORES):
        in_maps.append({
            "xpt": xpt,
            "wq": np.ascontiguousarray(wqf[:, c * HPC * D:(c + 1) * HPC * D]),
            "wk": np.ascontiguousarray(wkf[:, c * HPC * D:(c + 1) * HPC * D]),
            "sqk": sqk,
            "xt": np.ascontiguousarray(xtf[:, c * TLOC:(c + 1) * TLOC]),
            "w1": w1, "b1": b1, "w2": w2, "b2": b2, "w3": w3, "b3": b3,
        })
    return in_maps


def postprocess(results):
    weights = np.concatenate([results[c]["rw"] for c in range(NCORES)])
    weights = weights.astype(np.float32).reshape(T, 1)
    idx = np.concatenate([results[c]["ridx"] for c in range(NCORES)])
    idx = idx.astype(np.int32).reshape(T, 1)
    importance = np.zeros(T, dtype=np.float64)
    for c in range(NCORES):
        importance += results[c]["imp"].astype(np.float64)
    return weights, idx, importance.astype(np.float32)


def kernel(**inputs):
    from concourse.bass_utils import run_bass_kernel_spmd
    nc = _get_nc()
    in_maps = build_in_maps(inputs)
    res = run_bass_kernel_spmd(nc, in_maps, core_ids=list(range(NCORES)))
    return postprocess(res.results)


# AluOpType import at module scope for the builder
from concourse.alu_op_type import AluOpType  # noqa: E402
